# revision 32
# baseline (speedup 1.0000x reference)
"""Multi-head attention (B=4, S=2048, D=1024, H=16) on 8 Trainium2 cores.

Sharding: core c = (batch b = c//2, head-group g = c%2). Each core computes
8 heads' attention for one batch element plus the partial output projection
for its head-group's rows of Wo; the host sums the two partials per batch
and adds the bias.

Per-core kernel (all matmuls bf16, fp32 accumulation):
  xT      = x.T cast to bf16 (host-side prep)               [D, S]
  qT, kT  = Wg.T @ x.T (lhsT = W chunks, rhs = xT)          [G, S]
  v       = x @ Wv     (lhsT = xT chunks, rhs = Wv)         [S, G]
  per head-pair, per 512-wide q chunk, per 128-key block kb:
    sT    = k_h @ q_h.T (row-paired heads, K=64)            PSUM [128, 1024]
    pT    = exp: alternating ScalarE Exp / DVE Schraudolph  bf16 SBUF
    (3 key blocks later, so exp latency stays off the PE critical path:)
    ctx  += v_h.T @ pT  (col-paired heads, K=128)           PSUM [128, 512]
    den  += ones.T @ pT (col-paired, broadcast rows)        PSUM [128, 512]
  ctxT    = ctx * reciprocal(den)  (DVE)                    [G, S] bf16
  out     = ctxT.T @ Wo_g  -> fp32 partial to DRAM          [S, D]

Wq is pre-scaled by log2(e) on the host so the DVE Schraudolph exp is a
single tensor_scalar: int16(s2*16 + B16) bitcast as bf16 ~= exp(s2*ln2/8).
psum->sbuf copies run on ScalarE to keep the DVE free for exp.
"""

import numpy as np

B, S, D = 4, 2048, 1024
H, HD = 16, 64
NCORES = 8
G = D // 2  # head-group width per core (8 heads x 64)

LOG2E = float(np.log2(np.e))
# Schraudolph magic bias for bf16: i16 = s2*16 + B16, bitcast to bf16
# approximates exp(s2 * ln2/8) (s2 = raw_score * log2e, folded into Wq).
B16 = (127 - 0.0573) * 128.0
LAG = 3  # key blocks between scores/exp and ctx/den consumption

_BUILD_CACHE = {}


def build_mha(S=S, D=D, G=G, HD=HD):
    """Build the per-core Bass program. Returns the Bass object."""
    key = (S, D, G, HD)
    if key in _BUILD_CACHE:
        return _BUILD_CACHE[key]

    import concourse.bacc as bacc
    import concourse.mybir as mybir
    import concourse.tile as tile
    from contextlib import ExitStack

    FP32 = mybir.dt.float32
    BF16 = mybir.dt.bfloat16
    I16 = mybir.dt.int16
    MUL = mybir.AluOpType.mult
    ADD = mybir.AluOpType.add

    P = 128
    DC = D // P          # d_in chunks
    GC = G // P          # head-pair chunks
    SB = S // P          # seq blocks
    W = 512              # q-chunk width
    NW = S // W
    assert G % P == 0 and HD == 64 and S % 512 == 0

    nc = bacc.Bacc("TRN2", target_bir_lowering=False, debug=False)
    xt_d = nc.declare_dram_parameter("xt", [D, S], BF16, isOutput=False)
    wq_d = nc.declare_dram_parameter("wq", [D, G], BF16, isOutput=False)
    wk_d = nc.declare_dram_parameter("wk", [D, G], BF16, isOutput=False)
    wv_d = nc.declare_dram_parameter("wv", [D, G], BF16, isOutput=False)
    wo_d = nc.declare_dram_parameter("wo", [G, D], BF16, isOutput=False)
    out_d = nc.declare_dram_parameter("out", [S, D], FP32, isOutput=True)

    with tile.TileContext(nc) as tc, ExitStack() as ctx:
        const = ctx.enter_context(tc.tile_pool(name="const", bufs=1))
        wpool = ctx.enter_context(tc.tile_pool(name="wpool", bufs=1))
        big = ctx.enter_context(tc.tile_pool(name="big", bufs=1))
        ppool = ctx.enter_context(tc.tile_pool(name="ppool", bufs=10))
        norm = ctx.enter_context(tc.tile_pool(name="norm", bufs=4))
        outp = ctx.enter_context(tc.tile_pool(name="outp", bufs=6))
        pscore = ctx.enter_context(tc.tile_pool(name="pscore", bufs=3, space="PSUM"))
        pctx = ctx.enter_context(tc.tile_pool(name="pctx", bufs=1, space="PSUM"))
        pden = ctx.enter_context(tc.tile_pool(name="pden", bufs=1, space="PSUM"))

        ones_m = const.tile([P, HD], BF16)
        nc.gpsimd.memset(ones_m[:], 1.0)
        zbias = const.tile([P, 1], FP32)
        nc.gpsimd.memset(zbias[:], 0.0)

        # ---- loads: everything already bf16 / pre-transposed ----
        wq_sb = wpool.tile([P, DC, G], BF16)
        wk_sb = wpool.tile([P, DC, G], BF16)
        wv_sb = wpool.tile([P, DC, G], BF16)
        wo_sb = wpool.tile([P, GC, D], BF16)
        xt = big.tile([P, DC, S], BF16)

        # split the input load across both hardware DMA queues, sequenced to
        # match the projection emission order: the first group needs ALL of
        # wq plus x slice 0, so those two go first in parallel; later slices
        # land just ahead of their K-accumulation groups.
        def dma_x_slice(q, sc):
            for dc in range(DC):
                q.dma_start(
                    xt[:, dc, sc * 512:(sc + 1) * 512],
                    xt_d[dc * P:(dc + 1) * P, sc * 512:(sc + 1) * 512],
                )

        for c in range(DC):
            nc.scalar.dma_start(wq_sb[:, c, :], wq_d[c * P:(c + 1) * P, :])
        dma_x_slice(nc.sync, 0)
        dma_x_slice(nc.scalar, 1)
        for c in range(DC):
            nc.sync.dma_start(wk_sb[:, c, :], wk_d[c * P:(c + 1) * P, :])
        dma_x_slice(nc.sync, 2)
        dma_x_slice(nc.scalar, 3)
        for c in range(DC):
            nc.scalar.dma_start(wv_sb[:, c, :], wv_d[c * P:(c + 1) * P, :])
        for c in range(GC):
            nc.sync.dma_start(wo_sb[:, c, :], wo_d[c * P:(c + 1) * P, :])

        # ---- projections ----
        qt = big.tile([P, GC, S], BF16)
        kt = big.tile([P, GC, S], BF16)
        vp = big.tile([P, SB, G], BF16)

        def emit_proj_qk(g):
            for w_sb, dst in ((wq_sb, qt), (wk_sb, kt)):
                for sc in range(S // 512):
                    ps = pscore.tile([P, 512], FP32, tag="pscore", name="ps")
                    for dc in range(DC):
                        nc.tensor.matmul(
                            ps[:],
                            lhsT=w_sb[:, dc, g * P:(g + 1) * P],
                            rhs=xt[:, dc, sc * 512:(sc + 1) * 512],
                            start=(dc == 0),
                            stop=(dc == DC - 1),
                        )
                    nc.scalar.copy(dst[:, g, sc * 512:(sc + 1) * 512], ps[:])

        def emit_proj_v(sb):
            ps = pscore.tile([P, G], FP32, tag="pscore", name="ps")
            for dc in range(DC):
                nc.tensor.matmul(
                    ps[:],
                    lhsT=xt[:, dc, sb * P:(sb + 1) * P],
                    rhs=wv_sb[:, dc, :],
                    start=(dc == 0),
                    stop=(dc == DC - 1),
                )
            nc.scalar.copy(vp[:, sb, :], ps[:])

        # prologue: ALL projections as one PE-saturated phase. Keeping the
        # pscore pool free of filler tiles during attention keeps the
        # scores->exp->ctx ring at its design depth (3 bufs).
        emit_proj_qk(0)
        for sb in range(4):
            emit_proj_v(sb)
        for g in range(1, GC):
            emit_proj_qk(g)
        for sb in range(4, SB):
            emit_proj_v(sb)

        # ---- attention + normalization + output projection ----
        ctxT = big.tile([P, GC, S], BF16)
        EXP = mybir.ActivationFunctionType.Exp
        scale = float(np.log(2.0) / 8.0)  # Wq carries log2e

        for qw in range(NW):
            q0 = qw * W
            for p in range(GC):
                hA, hB = 2 * p, 2 * p + 1
                cacc = pctx.tile([P, 512], FP32, tag="pctx", name="cacc")
                dacc = pden.tile([P, 512], FP32, tag="pden", name="dacc")

                def emit_scores_exp(kb):
                    s = pscore.tile([P, 1024], FP32, tag="pscore", name="s")
                    nc.tensor.matmul(
                        s[:, 0:512],
                        lhsT=kt[0:64, p, kb * P:(kb + 1) * P],
                        rhs=qt[0:64, p, q0:q0 + 512],
                        start=True, stop=True,
                    )
                    nc.tensor.matmul(
                        s[:, 512:1024],
                        lhsT=kt[64:128, p, kb * P:(kb + 1) * P],
                        rhs=qt[64:128, p, q0:q0 + 512],
                        start=True, stop=True,
                    )
                    pt = ppool.tile([P, 1024], BF16, tag="ppool", name="pt")
                    if kb % 2 == 1:
                        # Schraudolph exp on the DVE: the int16 write of
                        # s2*16 + B16 IS bf16(exp(s2*ln2/8)) to ~2% ripple.
                        nc.vector.tensor_scalar(
                            pt[:].bitcast(I16), s[:], 16.0, B16, MUL, ADD,
                        )
                    else:
                        nc.scalar.activation(
                            pt[:], s[:], EXP, bias=zbias[:], scale=scale
                        )
                    return pt

                def emit_ctx(kb, pt):
                    first, last = kb == 0, kb == SB - 1
                    nc.tensor.matmul(
                        cacc[0:64, :],
                        lhsT=vp[:, kb, hA * HD:(hA + 1) * HD],
                        rhs=pt[:, 0:512], start=first, stop=last,
                        skip_group_check=True,
                    )
                    nc.tensor.matmul(
                        cacc[64:128, :],
                        lhsT=vp[:, kb, hB * HD:(hB + 1) * HD],
                        rhs=pt[:, 512:1024], start=first, stop=last,
                        skip_group_check=True,
                    )

                def emit_den(kb, pt):
                    first, last = kb == 0, kb == SB - 1
                    nc.tensor.matmul(
                        dacc[0:64, :], lhsT=ones_m[:],
                        rhs=pt[:, 0:512], start=first, stop=last,
                        skip_group_check=True,
                    )
                    nc.tensor.matmul(
                        dacc[64:128, :], lhsT=ones_m[:],
                        rhs=pt[:, 512:1024], start=first, stop=last,
                        skip_group_check=True,
                    )

                # macro-slot = 2 key blocks: batching S,S / C,C / D,D keeps
                # only two weight sets in flight at any moment, so every
                # LDWEIGHTS prefetches under the running matmul pair and the
                # pair cascade stays at the 512-cycle stream rate.
                pend = []
                for j in range(SB // 2):
                    pts = (emit_scores_exp(2 * j), emit_scores_exp(2 * j + 1))
                    pend.append((j, pts))
                    if len(pend) > 2:
                        jj, (ptA, ptB) = pend.pop(0)
                        emit_ctx(2 * jj, ptA)
                        emit_ctx(2 * jj + 1, ptB)
                        emit_den(2 * jj, ptA)
                        emit_den(2 * jj + 1, ptB)
                for jj, (ptA, ptB) in pend:
                    emit_ctx(2 * jj, ptA)
                    emit_ctx(2 * jj + 1, ptB)
                    emit_den(2 * jj, ptA)
                    emit_den(2 * jj + 1, ptB)

                # normalize promptly: recip/mult release the single-buffered
                # dacc/cacc banks for the next head-pair's accumulation.
                with tc.high_priority(offset=500_000):
                    rec = norm.tile([P, 512], FP32, tag="rec")
                    nc.vector.reciprocal_approx_fast(rec[:], dacc[:])
                    nc.vector.tensor_tensor(
                        ctxT[:, p, q0:q0 + 512], cacc[:], rec[:], MUL,
                    )

            # output projection phase for this q chunk
            NCW = min(512, D)
            if True:
                for sb in range(W // P):
                    row = q0 + sb * P
                    for nck in range(D // NCW):
                        po = pscore.tile([P, NCW], FP32, tag="pscore", name="po")
                        for g in range(GC):
                            nc.tensor.matmul(
                                po[:],
                                lhsT=ctxT[:, g, row:row + P],
                                rhs=wo_sb[:, g, nck * NCW:(nck + 1) * NCW],
                                start=(g == 0),
                                stop=(g == GC - 1),
                            )
                        ob = outp.tile([P, NCW], FP32, tag="ob")
                        nc.scalar.copy(ob[:], po[:])
                        nc.sync.dma_start(
                            out_d[row:row + P, nck * NCW:(nck + 1) * NCW], ob[:]
                        )

    nc.compile()
    _BUILD_CACHE[key] = nc
    return nc


def make_shards(x, Wq, Wk, Wv, Wo):
    """Split full inputs into 8 per-core input maps.

    Host-side layout prep only (dtype narrowing + transpose): the kernel
    consumes bf16 and x with the model dim on partitions. Wq additionally
    carries the log2(e) fold for the exp change-of-base.
    """
    import ml_dtypes
    BF = ml_dtypes.bfloat16
    x = np.asarray(x, dtype=np.float32)
    xt = np.ascontiguousarray(x.transpose(0, 2, 1)).astype(BF)  # [B, D, S]
    Wqb = (np.asarray(Wq, dtype=np.float32) * LOG2E).astype(BF)
    Wkb = np.asarray(Wk, dtype=np.float32).astype(BF)
    Wvb = np.asarray(Wv, dtype=np.float32).astype(BF)
    Wob = np.asarray(Wo, dtype=np.float32).astype(BF)
    shards = []
    for c in range(NCORES):
        b, g = divmod(c, 2)
        cs = slice(g * G, (g + 1) * G)
        shards.append({
            "xt": xt[b],
            "wq": np.ascontiguousarray(Wqb[:, cs]),
            "wk": np.ascontiguousarray(Wkb[:, cs]),
            "wv": np.ascontiguousarray(Wvb[:, cs]),
            "wo": np.ascontiguousarray(Wob[cs, :]),
        })
    return shards


def combine(results, bo):
    """Sum head-group partials per batch and add bias."""
    bo = np.asarray(bo, dtype=np.float32)
    outs = [results[c]["out"] for c in range(NCORES)]
    return np.stack([outs[2 * b] + outs[2 * b + 1] for b in range(B)]) + bo


def run_shards(shards, trace=False, **kw):
    from concourse.bass_utils import run_bass_kernel_spmd
    nc = build_mha()
    return run_bass_kernel_spmd(nc, shards, list(range(NCORES)), trace=trace, **kw)


def kernel(x, Wq, Wk, Wv, Wo, bo):
    res = run_shards(make_shards(x, Wq, Wk, Wv, Wo))
    return combine(res.results, bo)


# revision 34
# speedup vs baseline: 1.0012x; 1.0012x over previous
"""Multi-head attention (B=4, S=2048, D=1024, H=16) on 8 Trainium2 cores.

Sharding: core c = (batch b = c//2, head-group g = c%2). Each core computes
8 heads' attention for one batch element plus the partial output projection
for its head-group's rows of Wo; the host sums the two partials per batch
and adds the bias.

Per-core kernel (all matmuls bf16, fp32 accumulation):
  xT      = x.T cast to bf16 (host-side prep)               [D, S]
  qT, kT  = Wg.T @ x.T (lhsT = W chunks, rhs = xT)          [G, S]
  v       = x @ Wv     (lhsT = xT chunks, rhs = Wv)         [S, G]
  per head-pair, per 512-wide q chunk, per 128-key block kb:
    sT    = k_h @ q_h.T (row-paired heads, K=64)            PSUM [128, 1024]
    pT    = exp: alternating ScalarE Exp / DVE Schraudolph  bf16 SBUF
    (3 key blocks later, so exp latency stays off the PE critical path:)
    ctx  += v_h.T @ pT  (col-paired heads, K=128)           PSUM [128, 512]
    den  += ones.T @ pT (col-paired, broadcast rows)        PSUM [128, 512]
  ctxT    = ctx * reciprocal(den)  (DVE)                    [G, S] bf16
  out     = ctxT.T @ Wo_g  -> fp32 partial to DRAM          [S, D]

Wq is pre-scaled by log2(e) on the host so the DVE Schraudolph exp is a
single tensor_scalar: int16(s2*16 + B16) bitcast as bf16 ~= exp(s2*ln2/8).
psum->sbuf copies run on ScalarE to keep the DVE free for exp.
"""

import numpy as np

B, S, D = 4, 2048, 1024
H, HD = 16, 64
NCORES = 8
G = D // 2  # head-group width per core (8 heads x 64)

LOG2E = float(np.log2(np.e))
# Schraudolph magic bias for bf16: i16 = s2*16 + B16, bitcast to bf16
# approximates exp(s2 * ln2/8) (s2 = raw_score * log2e, folded into Wq).
B16 = (127 - 0.0573) * 128.0
LAG = 3  # key blocks between scores/exp and ctx/den consumption

_BUILD_CACHE = {}


def build_mha(S=S, D=D, G=G, HD=HD):
    """Build the per-core Bass program. Returns the Bass object."""
    key = (S, D, G, HD)
    if key in _BUILD_CACHE:
        return _BUILD_CACHE[key]

    import concourse.bacc as bacc
    import concourse.mybir as mybir
    import concourse.tile as tile
    from contextlib import ExitStack

    FP32 = mybir.dt.float32
    BF16 = mybir.dt.bfloat16
    I16 = mybir.dt.int16
    MUL = mybir.AluOpType.mult
    ADD = mybir.AluOpType.add

    P = 128
    DC = D // P          # d_in chunks
    GC = G // P          # head-pair chunks
    SB = S // P          # seq blocks
    W = 512              # q-chunk width
    NW = S // W
    assert G % P == 0 and HD == 64 and S % 512 == 0

    nc = bacc.Bacc("TRN2", target_bir_lowering=False, debug=False)
    xt_d = nc.declare_dram_parameter("xt", [D, S], BF16, isOutput=False)
    wq_d = nc.declare_dram_parameter("wq", [D, G], BF16, isOutput=False)
    wk_d = nc.declare_dram_parameter("wk", [D, G], BF16, isOutput=False)
    wv_d = nc.declare_dram_parameter("wv", [D, G], BF16, isOutput=False)
    wo_d = nc.declare_dram_parameter("wo", [G, D], BF16, isOutput=False)
    out_d = nc.declare_dram_parameter("out", [S, D], FP32, isOutput=True)

    with tile.TileContext(nc) as tc, ExitStack() as ctx:
        const = ctx.enter_context(tc.tile_pool(name="const", bufs=1))
        wpool = ctx.enter_context(tc.tile_pool(name="wpool", bufs=1))
        big = ctx.enter_context(tc.tile_pool(name="big", bufs=1))
        ppool = ctx.enter_context(tc.tile_pool(name="ppool", bufs=10))
        norm = ctx.enter_context(tc.tile_pool(name="norm", bufs=4))
        outp = ctx.enter_context(tc.tile_pool(name="outp", bufs=6))
        pscore = ctx.enter_context(tc.tile_pool(name="pscore", bufs=3, space="PSUM"))
        pctx = ctx.enter_context(tc.tile_pool(name="pctx", bufs=1, space="PSUM"))
        pden = ctx.enter_context(tc.tile_pool(name="pden", bufs=1, space="PSUM"))

        ones_m = const.tile([P, HD], BF16)
        nc.gpsimd.memset(ones_m[:], 1.0)
        zbias = const.tile([P, 1], FP32)
        nc.gpsimd.memset(zbias[:], 0.0)

        # ---- loads: everything already bf16 / pre-transposed ----
        wq_sb = wpool.tile([P, DC, G], BF16)
        wk_sb = wpool.tile([P, DC, G], BF16)
        wv_sb = wpool.tile([P, DC, G], BF16)
        wo_sb = wpool.tile([P, GC, D], BF16)
        xt = big.tile([P, DC, S], BF16)

        # split the input load across both hardware DMA queues, ordered so
        # each projection group's inputs land just before the PE needs them:
        # q/k weights first, then x column-slices alternating between queues.
        for c in range(DC):
            nc.scalar.dma_start(wq_sb[:, c, :], wq_d[c * P:(c + 1) * P, :])
            nc.sync.dma_start(wk_sb[:, c, :], wk_d[c * P:(c + 1) * P, :])
        for sc in range(S // 512):
            for dc in range(DC):
                q = nc.sync if dc % 2 == 0 else nc.scalar
                q.dma_start(
                    xt[:, dc, sc * 512:(sc + 1) * 512],
                    xt_d[dc * P:(dc + 1) * P, sc * 512:(sc + 1) * 512],
                )
        for c in range(DC):
            nc.scalar.dma_start(wv_sb[:, c, :], wv_d[c * P:(c + 1) * P, :])
        for c in range(GC):
            nc.sync.dma_start(wo_sb[:, c, :], wo_d[c * P:(c + 1) * P, :])

        # ---- projections ----
        qt = big.tile([P, GC, S], BF16)
        kt = big.tile([P, GC, S], BF16)
        vp = big.tile([P, SB, G], BF16)

        def emit_proj_qk(g):
            for w_sb, dst in ((wq_sb, qt), (wk_sb, kt)):
                for sc in range(S // 512):
                    ps = pscore.tile([P, 512], FP32, tag="pscore", name="ps")
                    for dc in range(DC):
                        nc.tensor.matmul(
                            ps[:],
                            lhsT=w_sb[:, dc, g * P:(g + 1) * P],
                            rhs=xt[:, dc, sc * 512:(sc + 1) * 512],
                            start=(dc == 0),
                            stop=(dc == DC - 1),
                        )
                    nc.scalar.copy(dst[:, g, sc * 512:(sc + 1) * 512], ps[:])

        def emit_proj_v(sb):
            ps = pscore.tile([P, G], FP32, tag="pscore", name="ps")
            for dc in range(DC):
                nc.tensor.matmul(
                    ps[:],
                    lhsT=xt[:, dc, sb * P:(sb + 1) * P],
                    rhs=wv_sb[:, dc, :],
                    start=(dc == 0),
                    stop=(dc == DC - 1),
                )
            nc.scalar.copy(vp[:, sb, :], ps[:])

        # prologue: ALL projections as one PE-saturated phase. Keeping the
        # pscore pool free of filler tiles during attention keeps the
        # scores->exp->ctx ring at its design depth (3 bufs).
        emit_proj_qk(0)
        for sb in range(4):
            emit_proj_v(sb)
        for g in range(1, GC):
            emit_proj_qk(g)
        for sb in range(4, SB):
            emit_proj_v(sb)

        # ---- attention + normalization + output projection ----
        ctxT = big.tile([P, GC, S], BF16)
        EXP = mybir.ActivationFunctionType.Exp
        scale = float(np.log(2.0) / 8.0)  # Wq carries log2e

        for qw in range(NW):
            q0 = qw * W
            for p in range(GC):
                hA, hB = 2 * p, 2 * p + 1
                cacc = pctx.tile([P, 512], FP32, tag="pctx", name="cacc")
                dacc = pden.tile([P, 512], FP32, tag="pden", name="dacc")

                def emit_scores_exp(kb):
                    s = pscore.tile([P, 1024], FP32, tag="pscore", name="s")
                    nc.tensor.matmul(
                        s[:, 0:512],
                        lhsT=kt[0:64, p, kb * P:(kb + 1) * P],
                        rhs=qt[0:64, p, q0:q0 + 512],
                        start=True, stop=True,
                    )
                    nc.tensor.matmul(
                        s[:, 512:1024],
                        lhsT=kt[64:128, p, kb * P:(kb + 1) * P],
                        rhs=qt[64:128, p, q0:q0 + 512],
                        start=True, stop=True,
                    )
                    pt = ppool.tile([P, 1024], BF16, tag="ppool", name="pt")
                    if kb % 2 == 1:
                        # Schraudolph exp on the DVE: the int16 write of
                        # s2*16 + B16 IS bf16(exp(s2*ln2/8)) to ~2% ripple.
                        nc.vector.tensor_scalar(
                            pt[:].bitcast(I16), s[:], 16.0, B16, MUL, ADD,
                        )
                    else:
                        nc.scalar.activation(
                            pt[:], s[:], EXP, bias=zbias[:], scale=scale
                        )
                    return pt

                def emit_ctx(kb, pt):
                    first, last = kb == 0, kb == SB - 1
                    nc.tensor.matmul(
                        cacc[0:64, :],
                        lhsT=vp[:, kb, hA * HD:(hA + 1) * HD],
                        rhs=pt[:, 0:512], start=first, stop=last,
                        skip_group_check=True,
                    )
                    nc.tensor.matmul(
                        cacc[64:128, :],
                        lhsT=vp[:, kb, hB * HD:(hB + 1) * HD],
                        rhs=pt[:, 512:1024], start=first, stop=last,
                        skip_group_check=True,
                    )

                def emit_den(kb, pt):
                    first, last = kb == 0, kb == SB - 1
                    nc.tensor.matmul(
                        dacc[0:64, :], lhsT=ones_m[:],
                        rhs=pt[:, 0:512], start=first, stop=last,
                        skip_group_check=True,
                    )
                    nc.tensor.matmul(
                        dacc[64:128, :], lhsT=ones_m[:],
                        rhs=pt[:, 512:1024], start=first, stop=last,
                        skip_group_check=True,
                    )

                # macro-slot = 2 key blocks: batching S,S / C,C / D,D keeps
                # only two weight sets in flight at any moment, so every
                # LDWEIGHTS prefetches under the running matmul pair and the
                # pair cascade stays at the 512-cycle stream rate.
                # alternate 4-wide ctx batches (even macros) and den
                # batches (odd macros): 2 weight-type transitions per macro
                # instead of 3, so more pairs issue at the stream rate.
                cq, dq = [], []
                for j in range(SB // 2):
                    pts = (emit_scores_exp(2 * j), emit_scores_exp(2 * j + 1))
                    cq.append((j, pts))
                    dq.append((j, pts))
                    if j % 2 == 0 and len(cq) > 2:
                        for jj, (ptA, ptB) in (cq.pop(0), cq.pop(0)):
                            emit_ctx(2 * jj, ptA)
                            emit_ctx(2 * jj + 1, ptB)
                    if j % 2 == 1 and len(dq) > 2:
                        for jj, (ptA, ptB) in (dq.pop(0), dq.pop(0)):
                            emit_den(2 * jj, ptA)
                            emit_den(2 * jj + 1, ptB)
                for jj, (ptA, ptB) in cq:
                    emit_ctx(2 * jj, ptA)
                    emit_ctx(2 * jj + 1, ptB)
                for jj, (ptA, ptB) in dq:
                    emit_den(2 * jj, ptA)
                    emit_den(2 * jj + 1, ptB)

                # normalize promptly: recip/mult release the single-buffered
                # dacc/cacc banks for the next head-pair's accumulation.
                with tc.high_priority(offset=500_000):
                    rec = norm.tile([P, 512], FP32, tag="rec")
                    nc.vector.reciprocal_approx_fast(rec[:], dacc[:])
                    nc.vector.tensor_tensor(
                        ctxT[:, p, q0:q0 + 512], cacc[:], rec[:], MUL,
                    )

            # output projection phase for this q chunk
            NCW = min(512, D)
            if True:
                for sb in range(W // P):
                    row = q0 + sb * P
                    for nck in range(D // NCW):
                        po = pscore.tile([P, NCW], FP32, tag="pscore", name="po")
                        for g in range(GC):
                            nc.tensor.matmul(
                                po[:],
                                lhsT=ctxT[:, g, row:row + P],
                                rhs=wo_sb[:, g, nck * NCW:(nck + 1) * NCW],
                                start=(g == 0),
                                stop=(g == GC - 1),
                            )
                        ob = outp.tile([P, NCW], FP32, tag="ob")
                        nc.scalar.copy(ob[:], po[:])
                        nc.sync.dma_start(
                            out_d[row:row + P, nck * NCW:(nck + 1) * NCW], ob[:]
                        )

    nc.compile()
    _BUILD_CACHE[key] = nc
    return nc


def make_shards(x, Wq, Wk, Wv, Wo):
    """Split full inputs into 8 per-core input maps.

    Host-side layout prep only (dtype narrowing + transpose): the kernel
    consumes bf16 and x with the model dim on partitions. Wq additionally
    carries the log2(e) fold for the exp change-of-base.
    """
    import ml_dtypes
    BF = ml_dtypes.bfloat16
    x = np.asarray(x, dtype=np.float32)
    xt = np.ascontiguousarray(x.transpose(0, 2, 1)).astype(BF)  # [B, D, S]
    Wqb = (np.asarray(Wq, dtype=np.float32) * LOG2E).astype(BF)
    Wkb = np.asarray(Wk, dtype=np.float32).astype(BF)
    Wvb = np.asarray(Wv, dtype=np.float32).astype(BF)
    Wob = np.asarray(Wo, dtype=np.float32).astype(BF)
    shards = []
    for c in range(NCORES):
        b, g = divmod(c, 2)
        cs = slice(g * G, (g + 1) * G)
        shards.append({
            "xt": xt[b],
            "wq": np.ascontiguousarray(Wqb[:, cs]),
            "wk": np.ascontiguousarray(Wkb[:, cs]),
            "wv": np.ascontiguousarray(Wvb[:, cs]),
            "wo": np.ascontiguousarray(Wob[cs, :]),
        })
    return shards


def combine(results, bo):
    """Sum head-group partials per batch and add bias."""
    bo = np.asarray(bo, dtype=np.float32)
    outs = [results[c]["out"] for c in range(NCORES)]
    return np.stack([outs[2 * b] + outs[2 * b + 1] for b in range(B)]) + bo


def run_shards(shards, trace=False, **kw):
    from concourse.bass_utils import run_bass_kernel_spmd
    nc = build_mha()
    return run_bass_kernel_spmd(nc, shards, list(range(NCORES)), trace=trace, **kw)


def kernel(x, Wq, Wk, Wv, Wo, bo):
    res = run_shards(make_shards(x, Wq, Wk, Wv, Wo))
    return combine(res.results, bo)


# revision 36
# speedup vs baseline: 1.0046x; 1.0034x over previous
"""Multi-head attention (B=4, S=2048, D=1024, H=16) on 8 Trainium2 cores.

Sharding: core c = (batch b = c//2, head-group g = c%2). Each core computes
8 heads' attention for one batch element plus the partial output projection
for its head-group's rows of Wo; the host sums the two partials per batch
and adds the bias.

Per-core kernel (all matmuls bf16, fp32 accumulation):
  xT      = x.T cast to bf16 (host-side prep)               [D, S]
  qT, kT  = Wg.T @ x.T (lhsT = W chunks, rhs = xT)          [G, S]
  v       = x @ Wv     (lhsT = xT chunks, rhs = Wv)         [S, G]
  per head-pair, per 512-wide q chunk, per 128-key block kb:
    sT    = k_h @ q_h.T (row-paired heads, K=64)            PSUM [128, 1024]
    pT    = exp: alternating ScalarE Exp / DVE Schraudolph  bf16 SBUF
    (3 key blocks later, so exp latency stays off the PE critical path:)
    ctx  += v_h.T @ pT  (col-paired heads, K=128)           PSUM [128, 512]
    den  += ones.T @ pT (col-paired, broadcast rows)        PSUM [128, 512]
  ctxT    = ctx * reciprocal(den)  (DVE)                    [G, S] bf16
  out     = ctxT.T @ Wo_g  -> fp32 partial to DRAM          [S, D]

Wq is pre-scaled by log2(e) on the host so the DVE Schraudolph exp is a
single tensor_scalar: int16(s2*16 + B16) bitcast as bf16 ~= exp(s2*ln2/8).
psum->sbuf copies run on ScalarE to keep the DVE free for exp.
"""

import numpy as np

B, S, D = 4, 2048, 1024
H, HD = 16, 64
NCORES = 8
G = D // 2  # head-group width per core (8 heads x 64)

LOG2E = float(np.log2(np.e))
# Schraudolph magic bias for bf16: i16 = s2*16 + B16, bitcast to bf16
# approximates exp(s2 * ln2/8) (s2 = raw_score * log2e, folded into Wq).
B16 = (127 - 0.0573) * 128.0
LAG = 3  # key blocks between scores/exp and ctx/den consumption

_BUILD_CACHE = {}


def build_mha(S=S, D=D, G=G, HD=HD):
    """Build the per-core Bass program. Returns the Bass object."""
    key = (S, D, G, HD)
    if key in _BUILD_CACHE:
        return _BUILD_CACHE[key]

    import concourse.bacc as bacc
    import concourse.mybir as mybir
    import concourse.tile as tile
    from contextlib import ExitStack

    FP32 = mybir.dt.float32
    BF16 = mybir.dt.bfloat16
    I16 = mybir.dt.int16
    MUL = mybir.AluOpType.mult
    ADD = mybir.AluOpType.add

    P = 128
    DC = D // P          # d_in chunks
    GC = G // P          # head-pair chunks
    SB = S // P          # seq blocks
    W = 512              # q-chunk width
    NW = S // W
    assert G % P == 0 and HD == 64 and S % 512 == 0

    nc = bacc.Bacc("TRN2", target_bir_lowering=False, debug=False)
    xt_d = nc.declare_dram_parameter("xt", [D, S], BF16, isOutput=False)
    wq_d = nc.declare_dram_parameter("wq", [D, G], BF16, isOutput=False)
    wk_d = nc.declare_dram_parameter("wk", [D, G], BF16, isOutput=False)
    wv_d = nc.declare_dram_parameter("wv", [D, G], BF16, isOutput=False)
    wo_d = nc.declare_dram_parameter("wo", [G, D], BF16, isOutput=False)
    out_d = nc.declare_dram_parameter("out", [S, D], FP32, isOutput=True)

    with tile.TileContext(nc) as tc, ExitStack() as ctx:
        big = ctx.enter_context(tc.tile_pool(name="big", bufs=1))
        ppool = ctx.enter_context(tc.tile_pool(name="ppool", bufs=8))
        norm = ctx.enter_context(tc.tile_pool(name="norm", bufs=8))
        pscore = ctx.enter_context(tc.tile_pool(name="pscore", bufs=3, space="PSUM"))
        pctx = ctx.enter_context(tc.tile_pool(name="pctx", bufs=1, space="PSUM"))
        pden = ctx.enter_context(tc.tile_pool(name="pden", bufs=1, space="PSUM"))

        ones_m = big.tile([P, HD], BF16)
        nc.gpsimd.memset(ones_m[:], 1.0)
        zbias = big.tile([P, 1], FP32)
        nc.gpsimd.memset(zbias[:], 0.0)

        # ---- loads: everything already bf16 / pre-transposed ----
        wq_sb = big.tile([P, DC, G], BF16)
        wk_sb = big.tile([P, DC, G], BF16)
        wv_sb = big.tile([P, DC, G], BF16)
        wo_sb = big.tile([P, GC, D], BF16)
        xt = big.tile([P, DC, S], BF16)

        # split the input load across both hardware DMA queues, ordered so
        # each projection group's inputs land just before the PE needs them:
        # q/k weights first, then x column-slices alternating between queues.
        for c in range(DC):
            nc.scalar.dma_start(wq_sb[:, c, :], wq_d[c * P:(c + 1) * P, :])
            nc.sync.dma_start(wk_sb[:, c, :], wk_d[c * P:(c + 1) * P, :])
        for sc in range(S // 512):
            for dc in range(DC):
                q = nc.sync if dc % 2 == 0 else nc.scalar
                q.dma_start(
                    xt[:, dc, sc * 512:(sc + 1) * 512],
                    xt_d[dc * P:(dc + 1) * P, sc * 512:(sc + 1) * 512],
                )
        for c in range(DC):
            nc.scalar.dma_start(wv_sb[:, c, :], wv_d[c * P:(c + 1) * P, :])
        for c in range(GC):
            nc.sync.dma_start(wo_sb[:, c, :], wo_d[c * P:(c + 1) * P, :])

        # ---- projections ----
        qt = big.tile([P, GC, S], BF16)
        kt = big.tile([P, GC, S], BF16)
        vp = big.tile([P, SB, G], BF16)

        def emit_proj_qk(g):
            for w_sb, dst in ((wq_sb, qt), (wk_sb, kt)):
                for sc in range(S // 512):
                    ps = pscore.tile([P, 512], FP32, tag="pscore", name="ps")
                    for dc in range(DC):
                        nc.tensor.matmul(
                            ps[:],
                            lhsT=w_sb[:, dc, g * P:(g + 1) * P],
                            rhs=xt[:, dc, sc * 512:(sc + 1) * 512],
                            start=(dc == 0),
                            stop=(dc == DC - 1),
                        )
                    nc.scalar.copy(dst[:, g, sc * 512:(sc + 1) * 512], ps[:])

        def emit_proj_v(sb):
            ps = pscore.tile([P, G], FP32, tag="pscore", name="ps")
            for dc in range(DC):
                nc.tensor.matmul(
                    ps[:],
                    lhsT=xt[:, dc, sb * P:(sb + 1) * P],
                    rhs=wv_sb[:, dc, :],
                    start=(dc == 0),
                    stop=(dc == DC - 1),
                )
            nc.scalar.copy(vp[:, sb, :], ps[:])

        # prologue: ALL projections as one PE-saturated phase. Keeping the
        # pscore pool free of filler tiles during attention keeps the
        # scores->exp->ctx ring at its design depth (3 bufs).
        emit_proj_qk(0)
        for sb in range(4):
            emit_proj_v(sb)
        for g in range(1, GC):
            emit_proj_qk(g)
        for sb in range(4, SB):
            emit_proj_v(sb)

        # ---- attention + normalization + output projection ----
        ctxT = big.tile([P, GC, S], BF16)
        EXP = mybir.ActivationFunctionType.Exp
        scale = float(np.log(2.0) / 8.0)  # Wq carries log2e

        for qw in range(NW):
            q0 = qw * W
            for p in range(GC):
                hA, hB = 2 * p, 2 * p + 1
                cacc = pctx.tile([P, 512], FP32, tag="pctx", name="cacc")
                dacc = pden.tile([P, 512], FP32, tag="pden", name="dacc")

                def emit_scores_exp(kb):
                    s = pscore.tile([P, 1024], FP32, tag="pscore", name="s")
                    nc.tensor.matmul(
                        s[:, 0:512],
                        lhsT=kt[0:64, p, kb * P:(kb + 1) * P],
                        rhs=qt[0:64, p, q0:q0 + 512],
                        start=True, stop=True,
                    )
                    nc.tensor.matmul(
                        s[:, 512:1024],
                        lhsT=kt[64:128, p, kb * P:(kb + 1) * P],
                        rhs=qt[64:128, p, q0:q0 + 512],
                        start=True, stop=True,
                    )
                    pt = ppool.tile([P, 1024], BF16, tag="ppool", name="pt")
                    if kb % 2 == 1:
                        # Schraudolph exp on the DVE: the int16 write of
                        # s2*16 + B16 IS bf16(exp(s2*ln2/8)) to ~2% ripple.
                        nc.vector.tensor_scalar(
                            pt[:].bitcast(I16), s[:], 16.0, B16, MUL, ADD,
                        )
                    else:
                        nc.scalar.activation(
                            pt[:], s[:], EXP, bias=zbias[:], scale=scale
                        )
                    return pt

                def emit_ctx(kb, pt):
                    first, last = kb == 0, kb == SB - 1
                    nc.tensor.matmul(
                        cacc[0:64, :],
                        lhsT=vp[:, kb, hA * HD:(hA + 1) * HD],
                        rhs=pt[:, 0:512], start=first, stop=last,
                        skip_group_check=True,
                    )
                    nc.tensor.matmul(
                        cacc[64:128, :],
                        lhsT=vp[:, kb, hB * HD:(hB + 1) * HD],
                        rhs=pt[:, 512:1024], start=first, stop=last,
                        skip_group_check=True,
                    )

                def emit_den(kb, pt):
                    first, last = kb == 0, kb == SB - 1
                    nc.tensor.matmul(
                        dacc[0:64, :], lhsT=ones_m[:],
                        rhs=pt[:, 0:512], start=first, stop=last,
                        skip_group_check=True,
                    )
                    nc.tensor.matmul(
                        dacc[64:128, :], lhsT=ones_m[:],
                        rhs=pt[:, 512:1024], start=first, stop=last,
                        skip_group_check=True,
                    )

                # macro-slot = 2 key blocks: batching S,S / C,C / D,D keeps
                # only two weight sets in flight at any moment, so every
                # LDWEIGHTS prefetches under the running matmul pair and the
                # pair cascade stays at the 512-cycle stream rate.
                pend = []
                for j in range(SB // 2):
                    pts = (emit_scores_exp(2 * j), emit_scores_exp(2 * j + 1))
                    pend.append((j, pts))
                    if len(pend) > 2:
                        jj, (ptA, ptB) = pend.pop(0)
                        emit_ctx(2 * jj, ptA)
                        emit_ctx(2 * jj + 1, ptB)
                        emit_den(2 * jj, ptA)
                        emit_den(2 * jj + 1, ptB)
                for jj, (ptA, ptB) in pend:
                    emit_ctx(2 * jj, ptA)
                    emit_ctx(2 * jj + 1, ptB)
                    emit_den(2 * jj, ptA)
                    emit_den(2 * jj + 1, ptB)

                # normalize promptly: recip/mult release the single-buffered
                # dacc/cacc banks for the next head-pair's accumulation.
                with tc.high_priority(offset=500_000):
                    rec = norm.tile([P, 512], FP32, tag="rec")
                    nc.vector.reciprocal_approx_fast(rec[:], dacc[:])
                    nc.vector.tensor_tensor(
                        ctxT[:, p, q0:q0 + 512], cacc[:], rec[:], MUL,
                    )

            # output projection phase for this q chunk
            NCW = min(512, D)
            if True:
                for sb in range(W // P):
                    row = q0 + sb * P
                    for nck in range(D // NCW):
                        po = pscore.tile([P, NCW], FP32, tag="pscore", name="po")
                        for g in range(GC):
                            nc.tensor.matmul(
                                po[:],
                                lhsT=ctxT[:, g, row:row + P],
                                rhs=wo_sb[:, g, nck * NCW:(nck + 1) * NCW],
                                start=(g == 0),
                                stop=(g == GC - 1),
                            )
                        ob = norm.tile([P, NCW], FP32, tag="ob")
                        nc.scalar.copy(ob[:], po[:])
                        nc.sync.dma_start(
                            out_d[row:row + P, nck * NCW:(nck + 1) * NCW], ob[:]
                        )

    nc.compile()
    _BUILD_CACHE[key] = nc
    return nc


def make_shards(x, Wq, Wk, Wv, Wo):
    """Split full inputs into 8 per-core input maps.

    Host-side layout prep only (dtype narrowing + transpose): the kernel
    consumes bf16 and x with the model dim on partitions. Wq additionally
    carries the log2(e) fold for the exp change-of-base.
    """
    import ml_dtypes
    BF = ml_dtypes.bfloat16
    x = np.asarray(x, dtype=np.float32)
    xt = np.ascontiguousarray(x.transpose(0, 2, 1)).astype(BF)  # [B, D, S]
    Wqb = (np.asarray(Wq, dtype=np.float32) * LOG2E).astype(BF)
    Wkb = np.asarray(Wk, dtype=np.float32).astype(BF)
    Wvb = np.asarray(Wv, dtype=np.float32).astype(BF)
    Wob = np.asarray(Wo, dtype=np.float32).astype(BF)
    shards = []
    for c in range(NCORES):
        b, g = divmod(c, 2)
        cs = slice(g * G, (g + 1) * G)
        shards.append({
            "xt": xt[b],
            "wq": np.ascontiguousarray(Wqb[:, cs]),
            "wk": np.ascontiguousarray(Wkb[:, cs]),
            "wv": np.ascontiguousarray(Wvb[:, cs]),
            "wo": np.ascontiguousarray(Wob[cs, :]),
        })
    return shards


def combine(results, bo):
    """Sum head-group partials per batch and add bias."""
    bo = np.asarray(bo, dtype=np.float32)
    outs = [results[c]["out"] for c in range(NCORES)]
    return np.stack([outs[2 * b] + outs[2 * b + 1] for b in range(B)]) + bo


def run_shards(shards, trace=False, **kw):
    from concourse.bass_utils import run_bass_kernel_spmd
    nc = build_mha()
    return run_bass_kernel_spmd(nc, shards, list(range(NCORES)), trace=trace, **kw)


def kernel(x, Wq, Wk, Wv, Wo, bo):
    res = run_shards(make_shards(x, Wq, Wk, Wv, Wo))
    return combine(res.results, bo)
